# revision 1
# baseline (speedup 1.0000x reference)
"""Dual-RoPE attention block (B=8, S=1024, 16 heads x 64) on 8 NeuronCores.

Sharding: data-parallel over batch, one batch element per core.

Per-core dataflow (all matmuls bf16 inputs, fp32 PSUM accumulation):
  - Q,K projected directly in head-transposed layout [d, s] (lhsT = w_qk.T),
    so the scores matmul needs no on-device transposes.
  - rotate_half done as a partition-swapped copy via SBUF->SBUF strip DMAs
    (compute engines cannot shift base partitions); RoPE is then 3 full-width
    vector multiplies per tile with presigned sin rows.
  - V projected in row layout [s, d]; each head's 65-column block in vext is
    [0.5-const | v], so the PV matmul's row 0 yields 0.5*sum_k(exp_scores).
  - softmax without max-subtraction (scores are O(10), exp safe in fp32).
  - 1/sum broadcast across the 64 head dims via a K=1 outer-product matmul
    (bc_col = 0.25 so the final scale is 0.5/sum = pass-averaging included).
  - per-head results assembled into [c, s] layout tiles by DMA, consumed as
    o_proj lhsT.
"""

import numpy as np
import ml_dtypes

B, S, DM = 8, 1024, 1024
NH, HD = 16, 64
HD1 = HD + 1
NC = 8                # cores

_CACHE = {}


def _build(rep=1, exp_n=1024, mm_n=512, dve_n=1024, swdma=True):
    key = ("nc", rep, exp_n, mm_n, dve_n, swdma)
    if key in _CACHE:
        return _CACHE[key]
    from concourse import bacc, mybir
    import concourse.tile as tile

    f32 = mybir.dt.float32
    bf16 = mybir.dt.bfloat16
    EXP = mybir.ActivationFunctionType.Exp

    nc = bacc.Bacc("TRN2", target_bir_lowering=False, debug=False,
                   enable_asserts=False, num_devices=NC)

    xT_d = nc.dram_tensor("xT", [DM, S], bf16, kind="ExternalInput").ap()
    wqkT_d = nc.dram_tensor("wqkT", [DM, 2 * DM], bf16, kind="ExternalInput").ap()
    wvT_d = nc.dram_tensor("wvT", [DM, DM], bf16, kind="ExternalInput").ap()
    woT_d = nc.dram_tensor("woT", [DM, DM], bf16, kind="ExternalInput").ap()
    trigC_d = nc.dram_tensor("trigC", [2, 128, S], bf16, kind="ExternalInput").ap()
    trigS_d = nc.dram_tensor("trigS", [2, 128, S], bf16, kind="ExternalInput").ap()
    out_d = nc.dram_tensor("out", [S, DM], f32, kind="ExternalOutput").ap()

    with tile.TileContext(nc) as tc:
        with (
            tc.tile_pool(name="persist", bufs=1) as pp,
            tc.tile_pool(name="ropeout", bufs=1) as ro,
        ):
            # constants / persistent tiles
            bc_col = pp.tile([1, HD1], bf16)
            nc.vector.memset(bc_col[0:1, :], 0.25)

            woT_sb = [pp.tile([128, DM], bf16, name=f"woT{i}") for i in range(8)]
            for i in range(8):
                nc.sync.dma_start(woT_sb[i][:], woT_d[i * 128:(i + 1) * 128, :])

            vext = [pp.tile([128, NH * HD1], bf16, name=f"vext{i}")
                    for i in range(8)]
            attn_b = [pp.tile([128, S], bf16, name=f"attnb{i}") for i in range(8)]

            # roped q/k for both passes: roped[pss][0..7]=q blocks, [8..15]=k
            roped = [[ro.tile([128, S], bf16, name=f"rope{p}_{t}")
                      for t in range(16)] for p in range(2)]

            for _rep in range(rep):
              # ---------- phase 1: projections + RoPE ----------
              with (
                  tc.tile_pool(name="w1", bufs=1) as wp,
                  tc.tile_pool(name="wv", bufs=2) as wvp,
                  tc.tile_pool(name="qkt", bufs=3) as qp,
                  tc.tile_pool(name="trig", bufs=1) as tp,
                  tc.tile_pool(name="ps1", bufs=3, space="PSUM") as ps1,
              ):
                  trigC_t = [tp.tile([128, S], bf16, name=f"trigC{p}")
                             for p in range(2)]
                  trigS_t = [tp.tile([128, S], bf16, name=f"trigS{p}")
                             for p in range(2)]
                  for p in range(2):
                      nc.sync.dma_start(trigC_t[p][:], trigC_d[p])
                      nc.sync.dma_start(trigS_t[p][:], trigS_d[p])

                  xT_sb = [wp.tile([128, S], bf16, name=f"xT{i}") for i in range(8)]
                  wqkT_sb = [wp.tile([128, 2 * DM], bf16, name=f"wqk{i}")
                             for i in range(8)]
                  for i in range(8):
                      nc.sync.dma_start(xT_sb[i][:], xT_d[i * 128:(i + 1) * 128, :])
                      nc.sync.dma_start(wqkT_sb[i][:],
                                        wqkT_d[i * 128:(i + 1) * 128, :])

                  # V in [s, c_v] row layout, strided into vext with 0.5 col 0
                  wvT_sb = []
                  for i in range(8):
                      wv = wvp.tile([128, DM], bf16, tag="wv", bufs=8,
                                    name=f"wv{i}")
                      nc.sync.dma_start(wv[:], wvT_d[i * 128:(i + 1) * 128, :])
                      wvT_sb.append(wv)
                  for sc in range(8):
                      ps = ps1.tile([128, S], f32, tag="ps1", bufs=3)
                      for dc in range(8):
                          for n in range(2):
                              nc.tensor.matmul(
                                  ps[:, n * 512:n * 512 + mm_n],
                                  xT_sb[dc][:, sc * 128:(sc + 1) * 128],
                                  wvT_sb[dc][:, n * 512:n * 512 + mm_n],
                                  start=(dc == 0), stop=(dc == 7))
                      vv = vext[sc][:].rearrange("p (h e) -> p h e", e=HD1)
                      nc.vector.tensor_copy(
                          vv[:, :, 1:HD1],
                          ps[:].rearrange("p (h e) -> p h e", e=HD))
                      nc.vector.memset(vv[:, :, 0:1], 2.0)

                  # Q,K in [c, s] layout: 16 chunks of 128 rows -> RoPE both passes
                  for cc in [0, 8, 1, 9, 2, 10, 3, 11, 4, 12, 5, 13, 6, 14, 7, 15]:
                      ps = ps1.tile([128, S], f32, tag="ps1", bufs=3)
                      for dc in range(8):
                          for n in range(2):
                              nc.tensor.matmul(
                                  ps[:, n * 512:n * 512 + mm_n],
                                  wqkT_sb[dc][:, cc * 128:(cc + 1) * 128],
                                  xT_sb[dc][:, n * 512:n * 512 + mm_n],
                                  start=(dc == 0), stop=(dc == 7))
                      qk = qp.tile([128, S], bf16, tag="qk", bufs=3,
                                   name=f"qk{cc}")
                      nc.vector.tensor_copy(qk[:], ps[:])
                      # partition-swapped copy (rotate_half layout) via DMA
                      sw = qp.tile([128, S], bf16, tag="sw", bufs=3,
                                   name=f"sw{cc}")
                      if swdma:
                          for hh in range(2):
                              for f in range(2):
                                  o0 = hh * 64 + f * 32
                                  i0 = hh * 64 + (1 - f) * 32
                                  nc.sync.dma_start(sw[o0:o0 + 32, :],
                                                    qk[i0:i0 + 32, :])
                      else:
                          nc.sync.dma_start(sw[0:32, :], qk[32:64, :])
                      for pss in range(2):
                          a = qp.tile([128, S], bf16, tag="ropeA", bufs=2,
                                      name=f"ropeA{cc}_{pss}")
                          bb = qp.tile([128, S], bf16, tag="ropeB", bufs=2,
                                       name=f"ropeB{cc}_{pss}")
                          nc.vector.tensor_mul(a[:, :dve_n], qk[:, :dve_n], trigC_t[pss][:, :dve_n])
                          nc.vector.tensor_mul(bb[:, :dve_n], sw[:, :dve_n], trigS_t[pss][:, :dve_n])
                          nc.vector.tensor_add(roped[pss][cc][:, :dve_n], a[:, :dve_n], bb[:, :dve_n])

              # ---------- phase 2: attention (both passes fused per head) ----
              with (
                  tc.tile_pool(name="expp", bufs=8) as ep,
                  tc.tile_pool(name="smal", bufs=3) as sp,
                  tc.tile_pool(name="scps", bufs=2, space="PSUM") as scp_pool,
                  tc.tile_pool(name="pvps", bufs=1, space="PSUM") as pvp_pool,
                  tc.tile_pool(name="bcps", bufs=1, space="PSUM") as bcp_pool,
              ):
                  for cc in range(8):
                      hE, hO = 2 * cc, 2 * cc + 1
                      pair_cts = []
                      for pss in range(2):
                          q1 = roped[pss][cc]
                          k1 = roped[pss][8 + cc]
                          pvps = [pvp_pool.tile([HD1, S], f32, tag="pvps",
                                                bufs=2,
                                                name=f"pvp{pss}_{2*cc+g}")
                                  for g in range(2)]
                          for kc in range(8):
                              for n in range(2):
                                  scp = scp_pool.tile(
                                      [128, S], f32, tag="scps", bufs=2,
                                      name=f"scp{pss}_{cc}_{kc}_{n}")
                                  for g, hh in ((0, 0), (1, 64)):
                                      nc.tensor.matmul(
                                          scp[:, g * 512:g * 512 + mm_n],
                                          k1[hh:hh + 64,
                                             kc * 128:(kc + 1) * 128],
                                          q1[hh:hh + 64,
                                             n * 512:n * 512 + mm_n],
                                          start=True, stop=True)
                                  es = ep.tile([128, S], bf16, tag="expS",
                                               bufs=5,
                                               name=f"es{pss}_{cc}_{kc}_{n}")
                                  nc.scalar.activation(es[:, :exp_n],
                                                       scp[:, :exp_n],
                                                       EXP, scale=0.125)
                                  for g, h in ((0, hE), (1, hO)):
                                      nc.tensor.matmul(
                                          pvps[g][:, n * 512:n * 512 + mm_n],
                                          vext[kc][:, h * HD1:(h + 1) * HD1],
                                          es[:, g * 512:g * 512 + mm_n],
                                          start=(kc == 0), stop=(kc == 7))

                          for g, h in ((0, hE), (1, hO)):
                              pvp = pvps[g]
                              pv_sb = sp.tile([HD1, S], bf16, tag="pvsb",
                                              bufs=4, name=f"pvsb{pss}_{h}")
                              nc.vector.tensor_copy(pv_sb[:, :dve_n],
                                                    pvp[:, :dve_n])
                              recf = sp.tile([1, S], f32, tag="recf",
                                             bufs=2, name=f"recf{pss}_{h}")
                              nc.vector.reciprocal_approx_fast(
                                  recf[0:1, :], pvp[0:1, :])
                              rec = sp.tile([1, S], bf16, tag="rec",
                                            bufs=3, name=f"rec{pss}_{h}")
                              with nc.allow_low_precision(
                                      reason="bf16 recip of softmax sums"):
                                  nc.vector.tensor_copy(rec[0:1, :],
                                                        recf[0:1, :])
                              bc_sb = sp.tile([HD1, S], bf16, tag="bcsb",
                                              bufs=2, name=f"bcsb{pss}_{h}")
                              nc.gpsimd.partition_broadcast(
                                  bc_sb[:, :], rec[0:1, :], channels=HD1)
                              ct = sp.tile([HD1, S], bf16, tag=f"ct{pss}{g}",
                                           bufs=2, name=f"ct{pss}_{h}")
                              nc.vector.tensor_mul(ct[:, :dve_n],
                                                   pv_sb[:, :dve_n],
                                                   bc_sb[:, :dve_n])
                              pair_cts.append(ct)

                      for g, h in ((0, hE), (1, hO)):
                          hh = (h % 2) * 64
                          ah = sp.tile([HD1, S], bf16, tag="ah", bufs=2,
                                       name=f"ah{h}")
                          nc.vector.tensor_add(ah[:, :dve_n],
                                               pair_cts[g][:, :dve_n],
                                               pair_cts[2 + g][:, :dve_n])
                          nc.sync.dma_start(attn_b[cc][hh:hh + 64, :],
                                            ah[1:HD1, :])

              # ---------- phase 3: output projection ----------
              with (
                  tc.tile_pool(name="ops", bufs=2, space="PSUM") as op_pool,
                  tc.tile_pool(name="outs", bufs=3) as out_pool,
              ):
                  for sc in range(8):
                      op = op_pool.tile([128, DM], f32, tag="op", bufs=2,
                                        name=f"op{sc}")
                      for cc in range(8):
                          for n in range(2):
                              nc.tensor.matmul(
                                  op[:, n * 512:n * 512 + mm_n],
                                  attn_b[cc][:, sc * 128:(sc + 1) * 128],
                                  woT_sb[cc][:, n * 512:n * 512 + mm_n],
                                  start=(cc == 0), stop=(cc == 7))
                      ob = out_pool.tile([128, DM], f32, tag="ob", bufs=3,
                                         name=f"ob{sc}")
                      nc.vector.tensor_copy(ob[:], op[:])
                      nc.sync.dma_start(out_d[sc * 128:(sc + 1) * 128, :], ob[:])

    nc.compile()
    _CACHE[key] = nc
    return nc


def _prep_inputs(hidden_states, cos, sin, w_qkv, w_o):
    bf = ml_dtypes.bfloat16
    xT = np.ascontiguousarray(
        hidden_states.transpose(0, 2, 1)).astype(bf)          # [B, DM, S]
    wqkT = np.ascontiguousarray(w_qkv[:2 * DM].T).astype(bf)  # [DM, 2DM]
    wvT = np.ascontiguousarray(w_qkv[2 * DM:].T).astype(bf)   # [DM, DM]
    woT = np.ascontiguousarray(w_o.T).astype(bf)              # [DM, DM]

    idx = np.arange(S).reshape(32, 32).T.reshape(-1)
    d = np.arange(128) % HD
    sign = np.where(d < 32, -1.0, 1.0).astype(np.float32)
    trigC = np.stack([
        np.ascontiguousarray(cos[:, d].T),
        np.ascontiguousarray(cos[idx][:, d].T),
    ]).astype(bf)                                             # [2, 128, S]
    trigS = np.stack([
        np.ascontiguousarray(sin[:, d].T) * sign[:, None],
        np.ascontiguousarray(sin[idx][:, d].T) * sign[:, None],
    ]).astype(bf)
    shared = {"wqkT": wqkT, "wvT": wvT, "woT": woT,
              "trigC": trigC, "trigS": trigS}
    return [{"xT": np.ascontiguousarray(xT[b]), **shared} for b in range(B)]


def _install_ntff_hook():
    import sys, types
    if "antenv.axon_hooks" in sys.modules:
        return
    try:
        from trn_agent_boot.trn_boot import _ntff_profile_via_ctypes
        hook = _ntff_profile_via_ctypes('/opt/axon/libaxon_pjrt.so')
    except Exception:
        hook = None
    mod = types.ModuleType("antenv.axon_hooks")
    mod.get_axon_ntff_profile_hook = lambda: hook
    mod.set_axon_ntff_profile_hook = lambda h: None
    sys.modules["antenv.axon_hooks"] = mod


def kernel(hidden_states, cos, sin, w_qkv, w_o, _trace=False, _tmpdir=None):
    from concourse import bass_utils
    if _trace:
        _install_ntff_hook()
    nc = _build()
    in_maps = _prep_inputs(np.asarray(hidden_states, np.float32),
                           np.asarray(cos, np.float32),
                           np.asarray(sin, np.float32),
                           np.asarray(w_qkv, np.float32),
                           np.asarray(w_o, np.float32))
    res = bass_utils.run_bass_kernel_spmd(
        nc, in_maps, core_ids=list(range(NC)),
        trace=_trace, tmpdir=_tmpdir)
    out = np.stack([np.asarray(res.results[b]["out"], np.float32)
                    for b in range(B)])
    kernel.last_exec_time_ns = res.exec_time_ns
    return out



# revision 2
# speedup vs baseline: 1.0212x; 1.0212x over previous
"""Dual-RoPE attention block (B=8, S=1024, 16 heads x 64) on 8 NeuronCores.

Single fused pipeline: the exp stream on ScalarE (the 1.1us/tile metronome,
256 tiles/core) starts ~25us in and every other engine pipelines around it.
PSUM budget (8 banks): 2x scores tile [128,1024]f32 (4 banks) + 2x PV tile
[65,1024]f32 (4 banks); projection chunks borrow scores buffers in
pass-tail windows.

Sharding: data-parallel over batch, one batch element per core.

Per-core dataflow (all matmuls bf16 inputs, fp32 PSUM accumulation):
  - head-pair 0's q/k projected from a tiny pre-sliced weight tensor
    (wqk0) so the first scores don't wait for the full 4MB wqkT load;
    inputs are spread across the SP/Activation/GpSimd DMA queues.
  - V-projection chunks interleaved with head-pair 0's first attention
    pass; both projection chunks of pair cc+1 run in cc's pass-0 tail so
    their RoPE (VectorE) overlaps pass 1 and the PE never idles long
    enough for the HAM clock gate to re-throttle.
  - scores row-tiled 2x (K=64; heads at partitions 0-63 / 64-127 run
    concurrently in the PE array).
  - PV with the 65-row trick: vext column blocks are [2.0-const | v], so PV
    row 0 yields 2*sum_k(exp); rec = recip(2 sum) folds pass-averaging.
  - softmax without max-subtraction (scores O(10), exp safe in fp32).
  - normalize split in two phases so the PV psum frees early; rec
    broadcast via gpsimd.partition_broadcast; output projection pipelined
    (per-sc partial over heads 0-6 while the last normalize completes).
"""

import numpy as np
import ml_dtypes

B, S, DM = 8, 1024, 1024
NH, HD = 16, 64
HD1 = HD + 1
NC = 8                # cores

# Schraudolph-on-DVE key-chunk assignment (kc values whose exp runs on
# VectorE instead of ScalarE). () disables.
DVE_KCS = ()
SCH_A = 184.6630
SCH_B = 16249.5

_CACHE = {}


def _build(dve_kcs=DVE_KCS):
    key = ("final", tuple(dve_kcs))
    if key in _CACHE:
        return _CACHE[key]
    from concourse import bacc, mybir
    import concourse.tile as tile

    f32 = mybir.dt.float32
    bf16 = mybir.dt.bfloat16
    i16 = mybir.dt.int16
    EXP = mybir.ActivationFunctionType.Exp
    MULT = mybir.AluOpType.mult
    ADD = mybir.AluOpType.add

    nc = bacc.Bacc("TRN2", target_bir_lowering=False, debug=False,
                   enable_asserts=False, num_devices=NC)

    xT_d = nc.dram_tensor("xT", [DM, S], bf16, kind="ExternalInput").ap()
    wqkT_d = nc.dram_tensor("wqkT", [DM, 2 * DM], bf16, kind="ExternalInput").ap()
    wqk0_d = nc.dram_tensor("wqk0", [128, 2048], bf16, kind="ExternalInput").ap()
    wvT_d = nc.dram_tensor("wvT", [DM, DM], bf16, kind="ExternalInput").ap()
    woT_d = nc.dram_tensor("woT", [DM, DM], bf16, kind="ExternalInput").ap()
    trig_d = nc.dram_tensor("trig", [128, 4 * S], bf16, kind="ExternalInput").ap()
    out_d = nc.dram_tensor("out", [S, DM], f32, kind="ExternalOutput").ap()

    with tile.TileContext(nc) as tc:
        with (
            tc.tile_pool(name="persist", bufs=1) as pp,
            tc.tile_pool(name="qkt", bufs=3) as qp,
            tc.tile_pool(name="expp", bufs=6) as ep,
            tc.tile_pool(name="smal", bufs=3) as sp,
            tc.tile_pool(name="bigps", bufs=2, space="PSUM") as bps,
            tc.tile_pool(name="pvps", bufs=1, space="PSUM") as pvp_pool,
        ):
            # ---------- persistent tiles + input DMA (ordered by need) -----
            xT_sb = [pp.tile([128, S], bf16, name=f"xT{i}") for i in range(8)]
            wqkT_sb = [pp.tile([128, 2 * DM], bf16, name=f"wqk{i}")
                       for i in range(8)]
            wvT_sb = [pp.tile([128, DM], bf16, name=f"wv{i}") for i in range(8)]
            woT_sb = [pp.tile([128, DM], bf16, name=f"woT{i}") for i in range(8)]
            trig_sb = pp.tile([128, 4 * S], bf16, name="trig")
            trigC_t = [trig_sb[:, p * S:(p + 1) * S] for p in range(2)]
            trigS_t = [trig_sb[:, (2 + p) * S:(3 + p) * S] for p in range(2)]
            vext = [pp.tile([128, NH * HD1], bf16, name=f"vext{i}")
                    for i in range(8)]
            attn_b = [pp.tile([128, S], bf16, name=f"attnb{i}") for i in range(8)]

            wqk0_sb = pp.tile([128, 2048], bf16, name="wqk0")
            nc.sync.dma_start(wqk0_sb[:], wqk0_d[:])
            nc.scalar.dma_start(trig_sb[:], trig_d[:])
            for i in range(8):
                eng = nc.sync if i % 2 == 0 else nc.scalar
                eng.dma_start(xT_sb[i][:], xT_d[i * 128:(i + 1) * 128, :])
            for i in range(8):
                nc.sync.dma_start(wqkT_sb[i][:],
                                  wqkT_d[i * 128:(i + 1) * 128, :])
            for i in range(8):
                # off the SP queue so the rotate-half swap strips (critical
                # path of the first rope) aren't stuck behind bulk weights
                nc.gpsimd.dma_start(wvT_sb[i][:], wvT_d[i * 128:(i + 1) * 128, :])
            for i in range(8):
                nc.gpsimd.dma_start(woT_sb[i][:], woT_d[i * 128:(i + 1) * 128, :])

            # roped q/k for both passes, double-buffered across cc:
            # roped[cc%2][pss][0]=q chunk, [1]=k chunk
            roped = [[[pp.tile([128, S], bf16, name=f"rope{par}_{p}_{t}")
                       for t in range(2)] for p in range(2)] for par in range(2)]

            def qk_proj_chunk(cc, which):
                """Project chunk `which` (0=q, 1=k) of head-pair cc into
                [c, s] layout and RoPE it for both passes."""
                wcol = cc + 8 * which
                ps = bps.tile([128, S], f32, tag="big", bufs=2,
                              name=f"qkps{cc}_{which}")
                for dc in range(8):
                    if cc == 0:
                        wsl = wqk0_sb[:, dc * 256 + which * 128:
                                      dc * 256 + (which + 1) * 128]
                    else:
                        wsl = wqkT_sb[dc][:, wcol * 128:(wcol + 1) * 128]
                    for n in range(2):
                        nc.tensor.matmul(
                            ps[:, n * 512:(n + 1) * 512],
                            wsl,
                            xT_sb[dc][:, n * 512:(n + 1) * 512],
                            start=(dc == 0), stop=(dc == 7))
                qk = qp.tile([128, S], bf16, tag="qk", bufs=2,
                             name=f"qk{cc}_{which}")
                nc.vector.tensor_copy(qk[:], ps[:])
                # partition-swapped (rotate_half) copy via SBUF DMA strips
                sw = qp.tile([128, S], bf16, tag="sw", bufs=2,
                             name=f"sw{cc}_{which}")
                for hh in range(2):
                    for f in range(2):
                        o0 = hh * 64 + f * 32
                        i0 = hh * 64 + (1 - f) * 32
                        nc.sync.dma_start(sw[o0:o0 + 32, :],
                                          qk[i0:i0 + 32, :])
                for pss in range(2):
                    a = qp.tile([128, S], bf16, tag="ropeA", bufs=1,
                                name=f"ropeA{cc}_{which}_{pss}")
                    bb = qp.tile([128, S], bf16, tag="ropeB", bufs=1,
                                 name=f"ropeB{cc}_{which}_{pss}")
                    nc.vector.tensor_mul(a[:], qk[:], trigC_t[pss][:])
                    nc.vector.tensor_mul(bb[:], sw[:], trigS_t[pss][:])
                    nc.vector.tensor_add(roped[cc % 2][pss][which][:],
                                         a[:], bb[:])

            def v_proj(sc):
                """Project V chunk sc (128 seq rows) into vext[sc] with the
                2.0-const column 0 per head."""
                ps = bps.tile([128, S], f32, tag="big", bufs=2,
                              name=f"vps{sc}")
                for dc in range(8):
                    for n in range(2):
                        nc.tensor.matmul(
                            ps[:, n * 512:(n + 1) * 512],
                            xT_sb[dc][:, sc * 128:(sc + 1) * 128],
                            wvT_sb[dc][:, n * 512:(n + 1) * 512],
                            start=(dc == 0), stop=(dc == 7))
                vv = vext[sc][:].rearrange("p (h e) -> p h e", e=HD1)
                nc.vector.tensor_copy(
                    vv[:, :, 1:HD1],
                    ps[:].rearrange("p (h e) -> p h e", e=HD))
                nc.vector.memset(vv[:, :, 0:1], 2.0)

            def attention(cc, pss, extras=None):
                """Both heads (2cc, 2cc+1) of pass pss.  extras: dict
                kc -> thunk emitted after that kc's exp (PE filler work)."""
                hE, hO = 2 * cc, 2 * cc + 1
                q1 = roped[cc % 2][pss][0]
                k1 = roped[cc % 2][pss][1]
                pvps = [pvp_pool.tile([HD1, S], f32, tag="pvps", bufs=2,
                                      name=f"pvp{pss}_{2 * cc + g}")
                        for g in range(2)]
                for kc in range(8):
                    es_t = []
                    for n in range(2):
                        scp = bps.tile([128, S], f32, tag="big", bufs=2,
                                       name=f"scp{pss}_{cc}_{kc}_{n}")
                        for g, hh in ((0, 0), (1, 64)):
                            nc.tensor.matmul(
                                scp[:, g * 512:(g + 1) * 512],
                                k1[hh:hh + 64, kc * 128:(kc + 1) * 128],
                                q1[hh:hh + 64, n * 512:(n + 1) * 512],
                                start=True, stop=True)
                        es = ep.tile([128, S], bf16, tag="expS", bufs=5,
                                     name=f"es{pss}_{cc}_{kc}_{n}")
                        if kc in dve_kcs:
                            nc.vector.tensor_scalar(
                                es[:].bitcast(i16), scp[:],
                                0.125 * SCH_A, SCH_B, MULT, ADD)
                        else:
                            nc.scalar.activation(es[:], scp[:], EXP,
                                                 scale=0.125)
                        es_t.append(es)
                    if extras and kc in extras:
                        extras[kc]()
                    # PV for this kc (stationary reused across n)
                    for g, h in ((0, hE), (1, hO)):
                        for n in range(2):
                            nc.tensor.matmul(
                                pvps[g][:, n * 512:(n + 1) * 512],
                                vext[kc][:, h * HD1:(h + 1) * HD1],
                                es_t[n][:, g * 512:(g + 1) * 512],
                                start=(kc == 0), stop=(kc == 7))

                # phase A of the normalize: free the PV psum tiles ASAP
                ab = []
                for g, h in ((0, hE), (1, hO)):
                    pvp = pvps[g]
                    pv_sb = sp.tile([HD1, S], bf16, tag="pvsb", bufs=3,
                                    name=f"pvsb{pss}_{h}")
                    nc.vector.tensor_copy(pv_sb[:], pvp[:])
                    recf = sp.tile([1, S], f32, tag="recf", bufs=2,
                                   name=f"recf{pss}_{h}")
                    nc.vector.reciprocal_approx_fast(recf[0:1, :],
                                                     pvp[0:1, :])
                    ab.append((pv_sb, recf))
                return ab

            def norm_phase_b(ab, cc, pss):
                hE, hO = 2 * cc, 2 * cc + 1
                cts = []
                for g, h in ((0, hE), (1, hO)):
                    pv_sb, recf = ab[g]
                    rec = sp.tile([1, S], bf16, tag="rec", bufs=2,
                                  name=f"rec{pss}_{h}")
                    with nc.allow_low_precision(
                            reason="bf16 recip of softmax sums"):
                        nc.vector.tensor_copy(rec[0:1, :], recf[0:1, :])
                    bc_sb = sp.tile([HD1, S], bf16, tag="bcsb", bufs=1,
                                    name=f"bcsb{pss}_{h}")
                    nc.gpsimd.partition_broadcast(bc_sb[:, :], rec[0:1, :],
                                                  channels=HD1)
                    ct = sp.tile([HD1, S], bf16, tag=f"ct{pss}{g}", bufs=1,
                                 name=f"ct{pss}_{h}")
                    nc.vector.tensor_mul(ct[:], pv_sb[:], bc_sb[:])
                    cts.append(ct)
                return cts

            # ---------- fused pipeline ----------
            qk_proj_chunk(0, 0)
            qk_proj_chunk(0, 1)
            v_proj(0)
            v_proj(1)

            for cc in range(8):
                ex0 = {kc: (lambda s=kc + 2: v_proj(s))
                       for kc in range(6)} if cc == 0 else None
                ab0 = attention(cc, 0, extras=ex0)
                if cc < 7:
                    qk_proj_chunk(cc + 1, 0)
                    qk_proj_chunk(cc + 1, 1)
                pair_cts = norm_phase_b(ab0, cc, 0)
                ab1 = attention(cc, 1)
                pair_cts += norm_phase_b(ab1, cc, 1)
                for g, h in ((0, 2 * cc), (1, 2 * cc + 1)):
                    hh = (h % 2) * 64
                    ah = sp.tile([HD1, S], bf16, tag="ah", bufs=2,
                                 name=f"ah{h}")
                    nc.vector.tensor_add(ah[:], pair_cts[g][:],
                                         pair_cts[2 + g][:])
                    nc.sync.dma_start(attn_b[cc][hh:hh + 64, :], ah[1:HD1, :])

            # ---------- output projection ----------
            # partial accumulation over heads 0-6 of the next sc chunk is
            # emitted before the cc=7 finisher of the current one, so the PE
            # works while the last head-pair's normalize chain completes.
            def oproj_partial(sc, op):
                for cc in range(7):
                    for n in range(2):
                        nc.tensor.matmul(
                            op[:, n * 512:(n + 1) * 512],
                            attn_b[cc][:, sc * 128:(sc + 1) * 128],
                            woT_sb[cc][:, n * 512:(n + 1) * 512],
                            start=(cc == 0), stop=False)

            def oproj_finish(sc, op):
                for n in range(2):
                    nc.tensor.matmul(
                        op[:, n * 512:(n + 1) * 512],
                        attn_b[7][:, sc * 128:(sc + 1) * 128],
                        woT_sb[7][:, n * 512:(n + 1) * 512],
                        start=False, stop=True)
                ob = sp.tile([128, DM], f32, tag="ob", bufs=2,
                             name=f"ob{sc}")
                nc.vector.tensor_copy(ob[:], op[:])
                eng = nc.sync if sc % 2 == 0 else nc.scalar
                eng.dma_start(out_d[sc * 128:(sc + 1) * 128, :], ob[:])

            ops = {}
            ops[0] = bps.tile([128, DM], f32, tag="big", bufs=2, name="op0")
            oproj_partial(0, ops[0])
            for sc in range(8):
                if sc + 1 < 8:
                    ops[sc + 1] = bps.tile([128, DM], f32, tag="big",
                                           bufs=2, name=f"op{sc + 1}")
                    oproj_partial(sc + 1, ops[sc + 1])
                oproj_finish(sc, ops.pop(sc))

    nc.compile()
    _CACHE[key] = nc
    return nc


def _prep_inputs(hidden_states, cos, sin, w_qkv, w_o):
    bf = ml_dtypes.bfloat16
    xT = np.ascontiguousarray(
        hidden_states.transpose(0, 2, 1)).astype(bf)          # [B, DM, S]
    wqkT = np.ascontiguousarray(w_qkv[:2 * DM].T).astype(bf)  # [DM, 2DM]
    wvT = np.ascontiguousarray(w_qkv[2 * DM:].T).astype(bf)   # [DM, DM]
    woT = np.ascontiguousarray(w_o.T).astype(bf)              # [DM, DM]

    idx = np.arange(S).reshape(32, 32).T.reshape(-1)
    d = np.arange(128) % HD
    sign = np.where(d < 32, -1.0, 1.0).astype(np.float32)
    trig = np.concatenate([
        cos[:, d].T, cos[idx][:, d].T,
        sin[:, d].T * sign[:, None], sin[idx][:, d].T * sign[:, None],
    ], axis=1).astype(bf)                                     # [128, 4S]
    wqk0 = np.ascontiguousarray(
        np.concatenate([wqkT[:, 0:128], wqkT[:, 1024:1152]], axis=1)
        .reshape(8, 128, 256).transpose(1, 0, 2).reshape(128, 2048))
    shared = {"wqkT": wqkT, "wqk0": wqk0, "wvT": wvT, "woT": woT,
              "trig": np.ascontiguousarray(trig)}
    return [{"xT": np.ascontiguousarray(xT[b]), **shared} for b in range(B)]


def _install_ntff_hook():
    import sys, types
    if "antenv.axon_hooks" in sys.modules:
        return
    try:
        from trn_agent_boot.trn_boot import _ntff_profile_via_ctypes
        hook = _ntff_profile_via_ctypes('/opt/axon/libaxon_pjrt.so')
    except Exception:
        hook = None
    mod = types.ModuleType("antenv.axon_hooks")
    mod.get_axon_ntff_profile_hook = lambda: hook
    mod.set_axon_ntff_profile_hook = lambda h: None
    sys.modules["antenv.axon_hooks"] = mod


def kernel(hidden_states, cos, sin, w_qkv, w_o, _trace=False, _tmpdir=None):
    from concourse import bass_utils
    if _trace:
        _install_ntff_hook()
    nc = _build()
    in_maps = _prep_inputs(np.asarray(hidden_states, np.float32),
                           np.asarray(cos, np.float32),
                           np.asarray(sin, np.float32),
                           np.asarray(w_qkv, np.float32),
                           np.asarray(w_o, np.float32))
    res = bass_utils.run_bass_kernel_spmd(
        nc, in_maps, core_ids=list(range(NC)),
        trace=_trace, tmpdir=_tmpdir)
    out = np.stack([np.asarray(res.results[b]["out"], np.float32)
                    for b in range(B)])
    kernel.last_exec_time_ns = res.exec_time_ns
    return out
